# revision 17
# baseline (speedup 1.0000x reference)
"""Trainium2 Bass kernel for nn_DiscretisedBNF (histogram binning MLP).

Math: the reference's per-bin CDF sum telescopes exactly (kl_{k+1} == kr_k
bit-identically, and cdf(kl_0) = cdf(kr_0) = 0 since those bounds are <= -1),
so

    sum_k [cdf(kr_k) - cdf(kl_k)] = cdf(kr_{K-1}) = 0.5*(1 + erf((0.875-mu_x)*inv))

with mu_x = mu/gamma - s*mu_eps, inv = 1/(sigma_x*sqrt(2)), sigma_x =
s*exp(ln_sigma_eps), s = sqrt((1-gamma)/gamma).  Rearranged for the chip,
with every per-column constant folded on the host:

    arg = (psA + M) * e              psA = 2^12 * (h @ W2A')     (PSUM)
    M   = 2^12*(mu*qm + qa + b2A)*EB   (host-precomputed, bf16)
    e   = exp(-2^-12*psB - ln(sqrt2) - 12ln2)                    (= E*2^-12)
    W2A'= W2A * EB,  EB = exp(-b2B)   (b2 of the B half folded as a
                                       multiplicative per-col factor)
    out = 0.5*erf(arg) + 0.5

Precision: both matmuls run in fp8 e4m3 with perf_mode=DoubleRow (2 fp8
weights/cell, ~2x PE rate, half the fp16 DMA bytes).  Scales keep everything
in e4m3's normal range (max +-240): x by 2^4, W1/W2 by 2^8.  The b1 bias and
the t-row of the concat([mu,t]) input are rank-1 terms, seeded into the mm1
PSUM accumulation by cheap rank-1 matmuls (so mm1's streamed contraction is
exactly D=4096 = 16 DoubleRow pairs, and the Lrelu needs no per-tile bias ->
two merged 1024-wide Lrelus).  Epilogue intermediates are bf16.  Simulated
end-to-end rel err: 9.8e-3 (deterministic seed) vs the 2e-2 gate.

Sharding: pure data parallel - batch dim (2048) split 256 rows per core;
weights replicated.  DoubleRow wants k-chunk PAIRS interleaved on the same
128 partitions (AP [128, 2, free]); the host packs accordingly.

Scheduling notes (from HW traces): the HAM clock gate needs a few us of
PE-busy to open and closes if the PE idles (rank-1 matmuls don't count);
mm1's own DoubleRow stream primes it.  The ACT engine reloads its function
table on every Exp<->Erf switch (1.5us!), so Exps run per-j while Erfs are
batched four-at-a-time (3 mid-stream loads total), with dummy activations
preloading the Lrelu/Exp tables during idle windows.  PSUM is managed as
[128,1024] two-bank tiles: mm1 uses 2 (4 m-chunks each), mm2 one psA + one
psB per j, so the epilogue runs 1024-wide ops (half the op count).  The last
j's epilogue chain is split across DVE+GpSimd halves to shorten the tail.
"""

import numpy as np
import ml_dtypes
from contextlib import ExitStack

import concourse.bass as bass
import concourse.mybir as mybir
from concourse.tile import TileContext
from concourse.tile_rust import add_dep_helper
from concourse.bass_utils import run_bass_kernel_spmd

B, D, H = 2048, 4096, 1024
NCORES = 8
BS = B // NCORES            # 256 batch rows per core
KP1 = 16                    # mm1 streamed contract pairs: 16*256 = 4096 = D
KC2 = H // 128              # 8 contract chunks for matmul2
KP2 = KC2 // 2              # 4 DoubleRow pairs
NJ = D // 512               # 8 output column groups of 512
LEAKY_SLOPE = 0.01
LN_SQRT2 = 0.34657359027997264
LN2 = 0.6931471805599453
SX = 2.0**4                 # x fp8 scale
SW = 2.0**8                 # W1/W2 fp8 scale

F8 = mybir.dt.float8e4
F16 = mybir.dt.float16
BF16 = mybir.dt.bfloat16
F32 = mybir.dt.float32
AF = mybir.ActivationFunctionType
OP = mybir.AluOpType
DR = mybir.MatmulPerfMode.DoubleRow

NPF8 = ml_dtypes.float8_e4m3
NPBF16 = ml_dtypes.bfloat16


def split_multi_waits(nc):
    """This container's walrus accepts at most ONE sync-wait per instruction
    (setupSyncWait: 'Too many sync wait commands').  Split any instruction
    carrying N>1 waits into N-1 single-wait NoOps on the same engine placed
    immediately before it."""
    cnt = 0
    sync_info_cls = None
    for f in nc.m.functions:
        for bb in f.blocks:
            out = []
            changed = False
            for inst in bb.instructions:
                si = inst.sync_info
                waits = list(si.on_wait) if si and si.on_wait else []
                if len(waits) > 1:
                    if sync_info_cls is None:
                        sync_info_cls = type(si)
                    for w in waits[:-1]:
                        nop = mybir.InstNoOp(name=f"waitsplit_{cnt}", ins=[], outs=[])
                        cnt += 1
                        nop.engine = inst.engine
                        nop.sync_info = sync_info_cls(on_wait=[w], on_update=[])
                        out.append(nop)
                    si.on_wait = waits[-1:]
                    changed = True
                out.append(inst)
            if changed:
                bb.instructions = out
    return cnt


def _lean_drain_and_barrier(self, tick_clock, wait_clock):
    """Replacement for TileContext._drain_and_barrier: drain + ONE barrier,
    skipping the ~7us semaphore-clear butterfly.  The Bass preamble re-clears
    every kernel semaphore at the start of each execution, and no sibling
    TileContext follows this one, so the tail clear is redundant.  The
    multi-wait drain is split later by split_multi_waits."""
    import concourse.tile as tile_mod

    nc = self.nc
    drain_inst = nc.sync.drain()
    wait_clock.add_sem_waits(
        drain_inst.ins, tile_mod.ScopedClock({None: tick_clock.global_clock})
    )
    # No all_engine_barrier: the SP drain above waits on every semaphore's
    # final tick (all engines' last work and all DMA completions), so SP
    # retires last and execution end implies everything finished.
    popped = nc._tile_sem_poison_stack.pop()
    assert popped is self._sem_poison


def _build():
    nc = bass.Bass()
    orig_drain = TileContext._drain_and_barrier
    TileContext._drain_and_barrier = _lean_drain_and_barrier
    try:
        _build_body(nc)
    finally:
        TileContext._drain_and_barrier = orig_drain

    split_multi_waits(nc)
    return nc


def _build_body(nc):
    xT = nc.dram_tensor("xT", [128, KP1, 2, BS], F8, kind="ExternalInput")
    w1 = nc.dram_tensor("w1", [128, KP1, 2, H], F8, kind="ExternalInput")
    w2 = nc.dram_tensor("w2", [128, NJ, KC2, 2, 512], F8, kind="ExternalInput")
    w1t = nc.dram_tensor("w1t", [1, H], F8, kind="ExternalInput")
    b1r = nc.dram_tensor("b1r", [1, H], F8, kind="ExternalInput")
    trow = nc.dram_tensor("trow", [1, BS], F8, kind="ExternalInput")
    mun = nc.dram_tensor("mun", [2, 128, D], BF16, kind="ExternalInput")
    outd = nc.dram_tensor("out", [BS, D], F16, kind="ExternalOutput")
    mun_r = mun.rearrange("h p d -> p h d")
    out_r = outd.rearrange("(h p) d -> p h d", p=128)

    with TileContext(nc) as tc, ExitStack() as ctx:
        const = ctx.enter_context(tc.tile_pool(name="const", bufs=1))
        xpool = ctx.enter_context(tc.tile_pool(name="xpool", bufs=1))
        w1pool = ctx.enter_context(tc.tile_pool(name="w1pool", bufs=4))
        hpool = ctx.enter_context(tc.tile_pool(name="hpool", bufs=1))
        w2pool = ctx.enter_context(tc.tile_pool(name="w2pool", bufs=6))
        eppool = ctx.enter_context(tc.tile_pool(name="eppool", bufs=4))
        outpool = ctx.enter_context(tc.tile_pool(name="outpool", bufs=3))
        pspool = ctx.enter_context(tc.tile_pool(name="pspool", bufs=4, space="PSUM"))

        # --- constants ---
        ones8 = const.tile([1, BS], F8, name="ones8")
        nc.vector.memset(ones8[:], 1.0)
        ones_row = const.tile([128, BS], F16, name="ones_row")
        nc.vector.memset(ones_row[:], 1.0)
        ones128 = const.tile([128, 128], F16, name="ones128")
        nc.vector.memset(ones128[:], 1.0)
        nln2_sb = const.tile([128, 1], F32, name="nln2_sb")
        nc.vector.memset(nln2_sb[:], -(LN_SQRT2 + 12.0 * LN2))
        scratch = const.tile([128, 1], F32, name="scratch")

        # preload the Lrelu ACT table (1.5us load, hidden under mm1)
        nc.scalar.activation(scratch[:], nln2_sb[:], AF.Lrelu, alpha=LEAKY_SLOPE)

        # short PE warm-up: dependency-free full-rank matmuls open the HAM
        # clock gate (needs ~3.5us of PE-busy; rank-1 seeds don't count)
        ps_warm = pspool.tile([128, 1024], F32, tag="ps", name="ps_warm")
        for _ in range(12):
            nc.tensor.matmul(
                ps_warm[:, :BS], ones128[:], ones_row[:], start=True, stop=True
            )

        # tiny rank-1 operand loads first on the SWDGE ring
        b1r_sb = const.tile([1, H], F8, name="b1r_sb")
        nc.gpsimd.dma_start(out=b1r_sb[:], in_=b1r[:])
        w1t_sb = const.tile([1, H], F8, name="w1t_sb")
        nc.gpsimd.dma_start(out=w1t_sb[:], in_=w1t[:])
        trow_sb = const.tile([1, BS], F8, name="trow_sb")
        nc.gpsimd.dma_start(out=trow_sb[:], in_=trow[:])

        # --- x^T resident, pair-packed; SWDGE ring (ahead of W2), split so
        # mm1's first pairs don't wait for the whole 1 MB.
        XT_PARTS = [2, 3, 4, 4, 3]  # pairs per part; front-load small
        xt_pair = {}
        q0 = 0
        for part, npair in enumerate(XT_PARTS):
            xt_q = xpool.tile(
                [128, max(XT_PARTS), 2, BS], F8, tag=f"xt{part}", name=f"xt_p{part}"
            )
            nc.gpsimd.dma_start(
                out=xt_q[:, :npair, :, :], in_=xT[:, q0 : q0 + npair, :, :]
            )
            for i in range(npair):
                xt_pair[q0 + i] = xt_q[:, i, :, :]
            q0 += npair
        assert q0 == KP1

        # --- matmul1: h^T = W1^T @ x^T, H on partitions, fp8 DoubleRow.
        # PSUM: two [128,1024] double-bank tiles, 4 m-chunks (256 cols) each.
        ps1_t = [
            pspool.tile([128, 1024], F32, tag="ps", name=f"ps1_t{i}")
            for i in range(2)
        ]

        def ps1(m):
            return ps1_t[m // 4][:, (m % 4) * BS : (m % 4 + 1) * BS]

        # rank-1 seeds: b1 (x2^12) and the t-row contribution t (x)W1[4096,:]
        for m in range(KC2):
            nc.tensor.matmul(
                ps1(m),
                b1r_sb[:, m * 128 : (m + 1) * 128],
                ones8[:],
                start=True,
                stop=False,
            )
        for m in range(KC2):
            nc.tensor.matmul(
                ps1(m),
                w1t_sb[:, m * 128 : (m + 1) * 128],
                trow_sb[:],
                start=False,
                stop=False,
            )

        W1_PARTS = [1, 1, 2, 2, 2, 2, 3, 3]  # pairs per group, Sync ring
        mm1_last = {}
        q = 0
        for g, npair in enumerate(W1_PARTS):
            w1g = w1pool.tile(
                [128, max(W1_PARTS), 2, H], F8, tag="w1t", name=f"w1g{g}"
            )
            nc.sync.dma_start(
                out=w1g[:, :npair, :, :], in_=w1[:, q : q + npair, :, :]
            )
            for i in range(npair):
                rhs = xt_pair[q]
                for m in range(KC2):
                    mm = nc.tensor.matmul(
                        ps1(m),
                        w1g[:, i, :, m * 128 : (m + 1) * 128],
                        rhs,
                        start=False,
                        stop=(q == KP1 - 1),
                        perf_mode=DR,
                    )
                mm1_last[q] = mm
                q += 1
        assert q == KP1

        # h fp8 at 2^4 scale, all 8 chunks in one [128, 8, 256] tile;
        # Lrelu(2^-8 * psum) = 2^4 * Lrelu(xW1 + b1): bias was seeded, so
        # TWO merged 1024-wide Lrelus cover all 8 chunks.
        h_all = hpool.tile([128, KC2, BS], F8, name="h_all")
        for i in range(2):
            nc.scalar.activation(
                h_all[:, 4 * i : 4 * (i + 1), :],
                ps1_t[i][:],
                AF.Lrelu,
                scale=2.0**-8,
                alpha=LEAKY_SLOPE,
            )
        # preload the Exp table while mm2's first matmuls run
        nc.scalar.activation(scratch[:], nln2_sb[:], AF.Exp)

        # --- matmul2 + fused epilogue ---
        w2ts, mu_js = {}, {}
        for j in range(NJ):
            w2t = w2pool.tile([128, KC2, 2, 512], F8, tag="w2", name=f"w2t{j}")
            pace = {0: 7, 1: 10, 2: 13, 3: 15}.get(j)
            dma = nc.gpsimd.dma_start(out=w2t[:], in_=w2[:, j])
            if pace is not None:
                add_dep_helper(dma.ins, mm1_last[pace].ins, True, "pace w2")
            w2ts[j] = w2t
            mu_j = eppool.tile([128, 1024], BF16, tag="mu", name=f"mu{j}")
            dma = nc.sync.dma_start(
                out=mu_j[:], in_=mun_r[:, :, j * 512 : (j + 1) * 512]
            )
            if j == 0:
                add_dep_helper(dma.ins, mm1_last[13].ins, True, "pace mu")
            mu_js[j] = mu_j

        def emit_matmuls(j, b_first):
            w2t = w2ts[j]
            psA = pspool.tile([128, 1024], F32, tag="ps", name=f"psA{j}")
            psB = pspool.tile([128, 1024], F32, tag="ps", name=f"psB{j}")
            dst = {0: psA, 1: psB}
            for h in ((1, 0) if b_first else (0, 1)):
                for qq in range(KP2):
                    for bh in range(2):
                        nc.tensor.matmul(
                            dst[h][:, bh * 512 : (bh + 1) * 512],
                            h_all[:, 2 * qq : 2 * qq + 2, bh * 128 : (bh + 1) * 128],
                            w2t[:, 2 * qq : 2 * qq + 2, h, :],
                            start=(qq == 0),
                            stop=(qq == KP2 - 1),
                            perf_mode=DR,
                        )
            return psA, psB

        g2s = {}
        pend_erf = []

        def flush_erfs(js):
            for j in js:
                o2 = outpool.tile([128, 1024], F16, tag="o", name=f"O{j}")
                if j == NJ - 1:
                    # split the last chain across ACT/DVE/GpSimd halves so
                    # the tail after the final matmul is ~half as deep
                    for bh in range(2):
                        sl = slice(bh * 512, (bh + 1) * 512)
                        r2 = eppool.tile([128, 512], BF16, tag="R", name=f"R{j}_{bh}")
                        nc.scalar.activation(r2[:], g2s[j][:, sl], AF.Erf)
                        eng = nc.gpsimd if bh == 0 else nc.vector
                        eng.tensor_scalar(o2[:, sl], r2[:], 0.5, 0.5, OP.mult, OP.add)
                        nc.sync.dma_start(
                            out=out_r[:, bh, j * 512 : (j + 1) * 512], in_=o2[:, sl]
                        )
                else:
                    r2 = eppool.tile([128, 1024], BF16, tag="Rw", name=f"R{j}")
                    nc.scalar.activation(r2[:], g2s[j][:], AF.Erf)
                    nc.gpsimd.tensor_scalar(o2[:], r2[:], 0.5, 0.5, OP.mult, OP.add)
                    nc.sync.dma_start(
                        out=out_r[:, :, j * 512 : (j + 1) * 512], in_=o2[:]
                    )

        ERF_FLUSH_AFTER = {3, 6}  # Erf batches [3,3,2]: 5 ACT table loads

        for j in range(NJ):
            last = j == NJ - 1
            psA, psB = emit_matmuls(j, b_first=last)
            s2 = eppool.tile([128, 1024], BF16, tag="S", name=f"S{j}")
            g2 = eppool.tile([128, 1024], BF16, tag="G", name=f"G{j}")
            e2 = eppool.tile([128, 1024], BF16, tag="E", name=f"E{j}")
            if last:
                # split the chain into 512-wide halves on the DVE (GpSimd
                # cannot read PSUM and is ~2x slower on tensor_tensor) so
                # the bh=0 half's Erf/out can start one op earlier.
                for bh in range(2):
                    sl = slice(bh * 512, (bh + 1) * 512)
                    nc.vector.tensor_tensor(
                        s2[:, sl], psA[:, sl], mu_js[j][:, sl], OP.add
                    )
                nc.scalar.activation(
                    e2[:], psB[:], AF.Exp, bias=nln2_sb[:], scale=-(2.0**-12)
                )
                for bh in range(2):
                    sl = slice(bh * 512, (bh + 1) * 512)
                    nc.vector.tensor_tensor(g2[:, sl], s2[:, sl], e2[:, sl], OP.mult)
            else:
                nc.vector.tensor_tensor(s2[:], psA[:], mu_js[j][:], OP.add)
                nc.scalar.activation(
                    e2[:], psB[:], AF.Exp, bias=nln2_sb[:], scale=-(2.0**-12)
                )
                nc.vector.tensor_tensor(g2[:], s2[:], e2[:], OP.mult)
            g2s[j] = g2
            pend_erf.append(j)
            if j in ERF_FLUSH_AFTER:
                flush_erfs(pend_erf[:-1])
                del pend_erf[:-1]
        flush_erfs(pend_erf)


_NC = None
_last_in_maps = None


def kernel(mu, t, gamma, W1, b1, W2, b2):
    global _NC
    if _NC is None:
        _NC = _build()
    nc = _NC

    f16 = np.float16
    f32 = np.float32

    def q8(a, scale):
        return np.clip(np.asarray(a, f32) * scale, -240.0, 240.0).astype(NPF8)

    # x^T = mu^T * 2^4 fp8, pair-packed [128, q, r, b]; the t column is a
    # rank-1 seed (trow (x) W1[4096,:])
    Xt = q8(mu, SX).T                      # (D, B)
    w1_np = np.ascontiguousarray(
        q8(W1[: D], SW).reshape(KP1, 2, 128, H).transpose(2, 0, 1, 3)
    )
    w1t_np = q8(W1[D], SW).reshape(1, H)
    b1r_np = q8(b1, SX * SW).reshape(1, H)  # b1 * 2^12

    b2_64 = np.asarray(b2, np.float64)
    b2A, b2B = b2_64[:D], b2_64[D:]
    EB = np.exp(-b2B)  # fold b2 of the B half as a per-col factor on W2A
    W2f = np.asarray(W2, f32).astype(np.float64)
    W2q = np.concatenate([W2f[:, :D] * EB[None, :], W2f[:, D:]], axis=1)
    # W2 pack [p, j, k, half, col] = W2q[k*128+p, half*D + j*512 + col]
    w2_np = np.ascontiguousarray(
        q8(W2q, SW).reshape(KC2, 128, 2, NJ, 512).transpose(1, 3, 0, 2, 4)
    )

    g64 = np.asarray(gamma, dtype=np.float64)[:, 0]
    s64 = np.sqrt((1.0 - g64) / g64)
    qm_t = -1.0 / (g64 * s64)
    qa_t = 0.875 / s64
    # M = 2^12 * (mu*qm + qa + b2A) * EB, bf16  (absmax ~4e5, well in range)
    mun2 = (
        (
            np.asarray(mu, np.float64) * qm_t[:, None]
            + qa_t[:, None]
            + b2A[None, :]
        )
        * EB[None, :]
        * (SX * SW)
    ).astype(NPBF16)
    t8 = q8(t, SX)  # (B, 1)

    in_maps = []
    for c in range(NCORES):
        sl = slice(c * BS, (c + 1) * BS)
        in_maps.append(
            {
                "xT": np.ascontiguousarray(
                    Xt[:, sl].reshape(KP1, 2, 128, BS).transpose(2, 0, 1, 3)
                ),
                "w1": w1_np,
                "w2": w2_np,
                "w1t": w1t_np,
                "b1r": b1r_np,
                "trow": np.ascontiguousarray(t8[sl, 0].reshape(1, BS)),
                "mun": np.ascontiguousarray(mun2[sl].reshape(2, 128, D)),
            }
        )

    global _last_in_maps
    _last_in_maps = in_maps

    res = run_bass_kernel_spmd(nc, in_maps, core_ids=list(range(NCORES)))
    return np.concatenate(
        [r["out"].astype(np.float32) for r in res.results], axis=0
    )
